# revision 15
# baseline (speedup 1.0000x reference)
"""Trainium2 Bass kernel for nn_LogicGatedSNN.

reference computation:
    w = ternary(synapse_states)            # {-1,0,+1}, threshold 1.0
    current = spike_input @ w.T            # [B, OUT]
    gated = current * (refractory<=0)
    spikes = (0.7*membrane + gated) >= adaptive_threshold

Sharding (8 cores): batch 2-way x out_features 4-way.
Each core: B_shard=4096, OUT_shard=512, K=IN=2048.

Host marshaling (lossless layout/dtype transforms only):
  - spikeT: spike_input.T as fp8e4m3 [IN, B_shard]  ({0,1} exact)
  - wT: synapse_states.T fp8e4m3 [IN, OUT_shard], nearest-round +
    4-mask fixup encoding that preserves the (>1)/(<-1) ternarize
    outcomes exactly; m-major layout within each k-pair so ternarize
    slivers and LDWEIGHTS reads are contiguous
  - nvec: membrane/threshold/refractory in [128, 4] per-partition layout

Device per core (v2 schedule, from trace analysis of v1 @74.8us):
  - fp8 weight staging halves the weight DMA bytes vs bf16; weights ride
    in pair-PAIR groups with 2KB per-partition runs (descriptor size
    drives the early HWDGE rate — 512B/1KB runs measured ~3-10x slower)
  - warmup memset on GPSIMD; 11 warmup matmuls bridge the HAM clock
    ramp to the first real matmul with no >3.4us PE idle (idle resets
    the ramp)
  - sync queue carries weight groups + n0 front chunks in deadline
    order, then n1 tail + n2..n7 blocks; scalar carries nvec + n1 front
    chunks and the mid-run output stores
  - phase A: k-pair-outer over 8 live PSUM banks, n1 lagging n0 by one
    k-pair (kills the v1 2.1us wait for the scalar-queue n1 chunk)
  - phases n=2..7: m-outer/t-inner, everything SBUF-resident
  - fused epilogue per psum tile: out_u8 = (current + bias_o) >= thr_o
    bias_o = 0.7*mem normally; +/-1e30 when refractory (always/never
    fire, chosen by the exact reference compare 0.7*mem >= thr)
  - last tile (n7,m3) runs as two 256-col accumulation groups so the
    final epilogue + output DMA are half-sized (shorter tail)
Output: out_u8 [OUT_shard, B_shard]; host transposes/casts/assembles.
"""
import os
import sys

sys.path.insert(0, "/opt/trn_rl_repo")
_HERE = os.path.dirname(os.path.abspath(__file__))
if _HERE not in sys.path:
    sys.path.insert(0, _HERE)

import numpy as np
import ml_dtypes

from concourse import bass, mybir
from concourse import tile
from concourse.bass_utils import run_bass_kernel_spmd

# ---- walrus CTRL sync-wait-slot workaround (inline, kernel.py must be
# self-contained). The TileContext tail drain carries one SyncWait per
# outstanding proc; this walrus build's CTRL template holds only 1.
import concourse.tile as _tile
from concourse.vector_clock import ScopedClock as _ScopedClock


def _patched_drain_and_barrier(self, tick_clock, wait_clock):
    nc = self.nc
    drain_inst = nc.sync.drain()
    wait_clock.add_sem_waits(
        drain_inst.ins, _ScopedClock({None: tick_clock.global_clock})
    )
    si = drain_inst.ins.sync_info
    if si is not None and si.on_wait and len(si.on_wait) > 1:
        waits = list(si.on_wait)
        si.on_wait = waits[:1]
        for i in range(1, len(waits)):
            extra = nc.sync.drain()
            esi = extra.ins.sync_info
            if esi is None:
                extra.ins.sync_info = mybir.SyncInfo(
                    on_wait=[waits[i]], on_update=[]
                )
            else:
                esi.on_wait = list(esi.on_wait or []) + [waits[i]]
    nc.all_engine_barrier()
    assert self.sems is not None
    popped = nc._tile_sem_poison_stack.pop()
    assert popped is self._sem_poison
    # semaphore clearing is skipped: this is the outermost (only) tile
    # scope and each kernel() invocation loads a fresh NEFF, so end-of-run
    # semaphore state is dead and nothing allocates semaphores after this


_tile.TileContext._drain_and_barrier = _patched_drain_and_barrier
# ---- end workaround


def _split_multi_waits(nc, max_waits=1):
    """This walrus build's instruction templates carry at most one
    semaphore wait. Hoist extra waits onto NoOps inserted just before the
    owning instruction on the same engine (engines execute their stream in
    order, so blocking semantics are identical)."""
    ctr = 0
    for f in nc.m.functions:
        for bb in f.blocks:
            new = []
            for inst in bb.instructions:
                si = inst.sync_info
                if si is not None and si.on_wait and len(si.on_wait) > max_waits:
                    waits = list(si.on_wait)
                    extra, keep = waits[:-max_waits], waits[-max_waits:]
                    for i in range(0, len(extra), max_waits):
                        ctr += 1
                        nop = mybir.InstNoOp(
                            name=f"{inst.name}-wsp{ctr}", ins=[], outs=[]
                        )
                        nop.engine = inst.engine
                        nop.bass_nofuse = True
                        nop.sync_info = mybir.SyncInfo(
                            on_wait=extra[i:i + max_waits], on_update=[]
                        )
                        new.append(nop)
                    si.on_wait = keep
                new.append(inst)
            bb.instructions = new


def _install_ntff_shim():
    """Provide antenv.axon_hooks (absent in this container) so
    run_bass_kernel_spmd(trace=True) can capture NTFF profiles via the
    loaded libaxon_pjrt.so C ABI."""
    import types
    import contextlib
    import ctypes

    try:
        from antenv import axon_hooks  # noqa: F401
        return
    except ImportError:
        pass
    so_path = "/opt/axon/libaxon_pjrt.so"
    if not os.path.exists(so_path):
        return
    lib = ctypes.CDLL(so_path)
    if not hasattr(lib, "axon_start_nrt_profile"):
        return
    lib.axon_start_nrt_profile.argtypes = [
        ctypes.POINTER(ctypes.c_int64), ctypes.c_size_t
    ]
    lib.axon_start_nrt_profile.restype = ctypes.c_int64
    lib.axon_stop_nrt_profile.argtypes = [ctypes.c_char_p]
    lib.axon_stop_nrt_profile.restype = ctypes.c_int64

    @contextlib.contextmanager
    def _hook(output_dir, device_ids):
        import jax

        jax.devices()
        if device_ids:
            ids = (ctypes.c_int64 * len(device_ids))(*device_ids)
            rc = lib.axon_start_nrt_profile(ids, len(device_ids))
        else:
            rc = lib.axon_start_nrt_profile(None, 0)
        if rc != 0:
            raise RuntimeError(f"axon_start_nrt_profile rc={rc}")
        try:
            yield
        finally:
            n = lib.axon_stop_nrt_profile(str(output_dir).encode())
            print(f"profile: {n} file(s) -> {output_dir}", file=sys.stderr)

    mod = types.ModuleType("antenv.axon_hooks")
    mod.get_axon_ntff_profile_hook = lambda: _hook
    mod.set_axon_ntff_profile_hook = lambda h: None
    sys.modules["antenv.axon_hooks"] = mod


_install_ntff_shim()

dt = mybir.dt

B, IN, OUT = 8192, 2048, 2048
PB, QO = 2, 4                 # batch blocks x out blocks = 8 cores
BS, OS = B // PB, OUT // QO   # 4096, 512 per-core shard sizes
KT = IN // 128                # 16 k-tiles
KT2 = KT // 2                 # 8 k-pairs (DoubleRow)
MT = OS // 128                # 4 m-tiles (out rows per core)
NB = 512                      # moving free dim per matmul
NT = BS // NB                 # 8 n-tiles
PAIRW = 2 * OS                # 1024 weight cols per k-pair
BIG = 1.0e30

LAST_EXEC_TIME_NS = None
LAST_TRACE = None

_BUILT = None


def _build():
    nc = bass.Bass()
    # host-marshaled layouts, fully contiguous per DMA block:
    #   spikeT: [NT, 128, KT*NB] n-block-major (8KB runs per partition row)
    #   wT:     [KT2//2, 128, 2*PAIRW] pair-PAIR-major fp8 (2KB runs per
    #           partition row — descriptor size drives early DMA rate),
    #           m-major within a pair (col = m*256 + j*128 + o)
    spikeT = nc.dram_tensor("spikeT", [NT * 128, KT * NB], dt.float8e4,
                            kind="ExternalInput")
    wT = nc.dram_tensor("wT", [(KT2 // 2) * 128, 2 * PAIRW], dt.float8e4,
                        kind="ExternalInput")
    nvec = nc.dram_tensor("nvec", [128, 3 * MT], dt.float32, kind="ExternalInput")
    # output as contiguous [m,n] blocks; host reassembles
    out = nc.dram_tensor("out_u8", [MT * NT * 128, NB], dt.uint8,
                         kind="ExternalOutput")

    AO = mybir.AluOpType
    DR = mybir.MatmulPerfMode.DoubleRow

    with tile.TileContext(nc) as tc:
        with tc.tile_pool(name="const", bufs=1) as cpool, \
             tc.tile_pool(name="wq", bufs=1) as wqpool, \
             tc.tile_pool(name="spk", bufs=1) as spkpool, \
             tc.tile_pool(name="wf", bufs=1) as wfpool, \
             tc.tile_pool(name="tern", bufs=8) as ternpool, \
             tc.tile_pool(name="outm", bufs=6) as outpool, \
             tc.tile_pool(name="tailo", bufs=1) as tailpool, \
             tc.tile_pool(name="ps", bufs=8, space="PSUM") as pspool:

            # resident ternary weights + spikes (fp8: exact for {0,1}
            # spikes and {-1,0,+1} weights)
            wq = wqpool.tile([128, KT2 * PAIRW], dt.float8e4)    # 8KB/part
            spk = spkpool.tile([128, NT * KT * NB], dt.float8e4)  # 64KB/part
            BLK = KT * NB   # 8192 columns per n-block in spk

            nv = cpool.tile([128, 3 * MT], dt.float32)
            wf = wfpool.tile([128, KT2 * PAIRW], dt.float8e4)    # 8KB/part
            wTv = wT[:].rearrange("(g p) c -> p g c", p=128)

            def w_group_dma(eng, g):
                # pair-PAIR g = k-pairs 2g, 2g+1; 2KB contiguous runs on
                # both sides
                eng.dma_start(
                    wf[:, g * 2 * PAIRW:(g + 1) * 2 * PAIRW],
                    wTv[:, g:g + 1, :],
                )

            def front_chunk(eng, n, col, width=2048):
                # spike cols within block n: col = k_tile*NB + b, so a
                # 2048-col chunk covers one k-PAIR-pair (pairs c, c+1)
                eng.dma_start(
                    spk[:, n * BLK + col:n * BLK + col + width],
                    spikeT[n * 128:(n + 1) * 128, col:col + width],
                )

            # ---- input DMA issue order ------------------------------------
            # All transfers use >=2KB per-partition runs (descriptor size
            # drives the early HWDGE rate). sync: weight groups + n0 front
            # interleaved by consumption deadline, then n1 tail, then the
            # resident blocks n2..7. scalar: nvec (tiny, needed ~20us), n1
            # front chunks, later the mid-run outs.
            w_group_dma(nc.sync, 0)            # k-pairs 0-1
            front_chunk(nc.sync, 0, 0)         # n0 k-pairs 0-1
            w_group_dma(nc.sync, 1)            # k-pairs 2-3
            w_group_dma(nc.sync, 2)            # k-pairs 4-5
            front_chunk(nc.sync, 0, 2048)      # n0 k-pairs 2-3
            w_group_dma(nc.sync, 3)            # k-pairs 6-7
            front_chunk(nc.sync, 0, 4096)      # n0 k-pairs 4-5
            front_chunk(nc.sync, 0, 6144)      # n0 k-pairs 6-7
            front_chunk(nc.sync, 1, 4096)      # n1 k-pairs 4-5
            front_chunk(nc.sync, 1, 6144)      # n1 k-pairs 6-7
            for n in range(2, NT):
                nc.sync.dma_start(
                    spk[:, n * BLK:(n + 1) * BLK],
                    spikeT[n * 128:(n + 1) * 128, :],
                )
            # scalar queue
            nc.scalar.dma_start(nv[:], nvec[:])
            front_chunk(nc.scalar, 1, 0)       # n1 k-pairs 0-1
            front_chunk(nc.scalar, 1, 2048)    # n1 k-pairs 2-3

            # ---- PE warmup: short dummy chain starts the HAM clock ramp
            # while the first loads are in flight (memset on GPSIMD so DVE
            # can start ternarizing the moment pair-0 weights land)
            wrm = cpool.tile([128, 640], dt.float8e4)
            nc.gpsimd.memset(wrm[:], 0.0)
            # per-partition bias constants for the ACT Sign ternarize
            bm5 = cpool.tile([128, 1], dt.float32)
            nc.gpsimd.memset(bm5[:], -0.5)
            bp5 = cpool.tile([128, 1], dt.float32)
            nc.gpsimd.memset(bp5[:], 0.5)
            pswrm = pspool.tile([128, NB], dt.float32, tag="ps")
            NWRM = 9
            for i in range(NWRM):
                nc.tensor.matmul(
                    pswrm[:], wrm[:, 0:128], wrm[:, 128:640],
                    start=(i == 0), stop=(i == NWRM - 1),
                )

            # ---- per-neuron epilogue scalars (DVE, tiny; nvec rides first
            # on the fast sync queue so these unblock ~7.3us even if the
            # scheduler orders them before the pair-0 ternarize)
            mem = nv[:, 0:MT]
            thr = nv[:, MT:2 * MT]
            refr = nv[:, 2 * MT:3 * MT]

            b07 = cpool.tile([128, MT], dt.float32)
            nc.vector.tensor_scalar(b07[:], mem, 0.7, None, AO.mult)
            # cond = (0.7*mem >= thr)  — exact reference compare for
            # refractory neurons (their new_v is exactly 0.7*mem)
            cond = cpool.tile([128, MT], dt.float32)
            nc.vector.tensor_tensor(cond[:], b07[:], thr, AO.is_ge)
            # bigsel = cond*2BIG - BIG  in {-BIG, +BIG}
            bigsel = cpool.tile([128, MT], dt.float32)
            nc.vector.tensor_scalar(bigsel[:], cond[:], 2.0 * BIG, -BIG,
                                    AO.mult, AO.add)
            # sel = refractory? (refr > 0)
            sel = cpool.tile([128, MT], dt.float32)
            nc.vector.tensor_scalar(sel[:], refr, 0.0, None, AO.is_gt)
            # bias = b07 + sel * (bigsel - b07)
            dvt = cpool.tile([128, MT], dt.float32)
            nc.vector.tensor_sub(dvt[:], bigsel[:], b07[:])
            nc.vector.tensor_mul(dvt[:], dvt[:], sel[:])
            bias = cpool.tile([128, MT], dt.float32)
            nc.vector.tensor_add(bias[:], b07[:], dvt[:])
            # doubled copies to match the {-2,0,+2} weights (x2 exact)
            thr2 = cpool.tile([128, MT], dt.float32)
            nc.vector.tensor_scalar(thr2[:], thr, 2.0, None, AO.mult)
            bias2 = cpool.tile([128, MT], dt.float32)
            nc.vector.tensor_scalar(bias2[:], bias[:], 2.0, None, AO.mult)

            # ---- streaming ternarize to DOUBLED weights {-2,0,+2} (thr and
            # bias are doubled to match — exact: x2 is a pure exponent
            # shift in fp32). Measured: DVE fp8 compare ops run ~1.6x
            # SLOWER than bf16 (684ns vs 419ns per [128,1024] op), so a
            # DVE-only chain (~17us serial) gates phase A. Instead the
            # otherwise-idle ACT engine computes two Sign activations per
            # half-pair — sign(0.5s-0.5) + sign(0.5s+0.5) = 2*ternary(s)
            # exactly, because the host encoding never produces exact
            # +/-1.0 — and DVE only does one cheap add per half.
            AF = mybir.ActivationFunctionType
            sga = wfpool.tile([128, KT2 * PAIRW], dt.float8e4)
            sgb = wfpool.tile([128, KT2 * PAIRW], dt.float8e4)

            # pair 0 entirely on DVE in per-m slivers (3 small ops each)
            # so the first LDWEIGHTS unblocks as early as possible
            for m in range(MT):
                lo = m * 256
                sl = wf[:, lo:lo + 256]
                a2 = ternpool.tile([128, 256], dt.float8e4, tag="neg")
                b2 = ternpool.tile([128, 256], dt.float8e4, tag="neg")
                nc.vector.tensor_scalar(a2[:], sl, 1.0, 2.0,
                                        AO.is_gt, AO.mult)
                nc.vector.tensor_scalar(b2[:], sl, -1.0, 2.0,
                                        AO.is_gt, AO.mult)
                nc.vector.scalar_tensor_tensor(
                    wq[:, lo:lo + 256], a2[:], -2.0, b2[:], AO.add, AO.add,
                )
            # pairs 1..7: ACT signs + DVE add, half-pair granularity
            for t in range(1, KT2):
                for h in range(2):
                    lo = t * PAIRW + h * 512
                    sl = wf[:, lo:lo + 512]
                    nc.scalar.activation(sga[:, lo:lo + 512], sl, AF.Sign,
                                         bias=bm5[:, 0:1], scale=0.5)
                    nc.scalar.activation(sgb[:, lo:lo + 512], sl, AF.Sign,
                                         bias=bp5[:, 0:1], scale=0.5)
                    nc.vector.tensor_tensor(
                        wq[:, lo:lo + 512], sga[:, lo:lo + 512],
                        sgb[:, lo:lo + 512], AO.add,
                    )

            # stationary view: [p, pair, m, j, o] — LDW for (t,m) reads a
            # contiguous 256-col range as a [128, 2, 128] AP
            wqv = wq[:].rearrange("p (t m j o) -> p t m j o",
                                  t=KT2, m=MT, j=2)
            spknv = [
                spk[:, n * BLK:(n + 1) * BLK].rearrange(
                    "p (t b) -> p t b", t=KT)
                for n in range(NT)
            ]

            def mm(ps_ap, t, m, n, start, stop, cols=None):
                mv = spknv[n][:, 2 * t:2 * t + 2, :]
                if cols is not None:
                    mv = spknv[n][:, 2 * t:2 * t + 2, cols[0]:cols[1]]
                nc.tensor.matmul(
                    ps_ap, wqv[:, t, m], mv,
                    start=start, stop=stop, perf_mode=DR,
                )

            def epilogue(ps, m, n):
                # spikes = (current + bias_o) >= thr_o — single fused DVE
                # op from PSUM
                om = outpool.tile([128, NB], dt.uint8)
                nc.vector.tensor_scalar(
                    om[:], ps[:],
                    bias2[:, m:m + 1], thr2[:, m:m + 1],
                    AO.add, AO.is_ge,
                )
                blk = (m * NT + n) * 128
                # last-phase outs ride the (by then idle) sync queue so the
                # final store isn't queued behind earlier outs
                eng = nc.sync if n == NT - 1 else nc.scalar
                eng.dma_start(out[blk:blk + 128, :], om[:])

            # ---- phase A: n-blocks 0..1, k-pair-outer over 8 live banks,
            # n1 lagging n0 by one k-pair ------------------------------------
            psA = []
            for n in range(2):
                for m in range(MT):
                    ps = pspool.tile([128, NB], dt.float32,
                                     name=f"psA_{n}_{m}", tag="ps")
                    psA.append(ps)
            for t in range(KT2):
                for m in range(MT):
                    mm(psA[m][:], t, m, 0,
                       start=(t == 0), stop=(t == KT2 - 1))
                if t >= 1:
                    tl = t - 1
                    for m in range(MT):
                        mm(psA[MT + m][:], tl, m, 1,
                           start=(tl == 0), stop=False)
            for m in range(MT):
                epilogue(psA[m], m, 0)
            for m in range(MT):
                mm(psA[MT + m][:], KT2 - 1, m, 1, start=False, stop=True)
            for m in range(MT):
                epilogue(psA[MT + m], m, 1)

            # ---- phases n=2..7: everything resident, m-outer/t-inner ------
            for n in range(2, NT):
                for m in range(MT):
                    if n == NT - 1 and m == MT - 1:
                        break  # handled below as two half-width groups
                    ps = pspool.tile([128, NB], dt.float32,
                                     name=f"ps_{n}_{m}", tag="ps")
                    for t in range(KT2):
                        mm(ps[:], t, m, n,
                           start=(t == 0), stop=(t == KT2 - 1))
                    epilogue(ps, m, n)

            # ---- last tile (n7, m3): two 256-col accumulation groups so
            # the final epilogue + store are half-sized
            psL = pspool.tile([128, NB], dt.float32, name="ps_last", tag="ps")
            mL, nL = MT - 1, NT - 1
            blkL = (mL * NT + nL) * 128
            for h in range(2):
                cols = (h * 256, (h + 1) * 256)
                for t in range(KT2):
                    mm(psL[:, cols[0]:cols[1]], t, mL, nL,
                       start=(t == 0), stop=(t == KT2 - 1), cols=cols)
                omh = tailpool.tile([128, 256], dt.uint8, name=f"om_tail{h}")
                nc.vector.tensor_scalar(
                    omh[:], psL[:, cols[0]:cols[1]],
                    bias2[:, mL:mL + 1], thr2[:, mL:mL + 1],
                    AO.add, AO.is_ge,
                )
                eng = nc.scalar if h == 0 else nc.sync
                eng.dma_start(
                    out[blkL:blkL + 128, cols[0]:cols[1]], omh[:]
                )

    _split_multi_waits(nc)
    return nc


def _get_built():
    global _BUILT
    if _BUILT is None:
        _BUILT = _build()
    return _BUILT


def kernel(spike_input, synapse_states, membrane_potential,
           adaptive_threshold, refractory_count):
    global LAST_EXEC_TIME_NS, LAST_TRACE
    nc = _get_built()

    spike8 = spike_input.astype(ml_dtypes.float8_e4m3)   # [B, IN], {0,1}
    # compare-preserving fp8 weight encoding: nearest-round to e4m3, then
    # fix the few values whose (>1)/(<-1) outcome rounding would flip.
    # The encoding never produces exactly +/-1.0, so the device's strict
    # sign compares at the +/-1 boundaries are exact.
    s32 = np.ascontiguousarray(synapse_states.astype(np.float32, copy=False).T)
    q = s32.astype(ml_dtypes.float8_e4m3)
    qf = q.astype(np.float32)
    mid = (s32 <= 1.0) & (s32 >= -1.0)
    q[(s32 > 1.0) & (qf <= 1.0)] = ml_dtypes.float8_e4m3(1.0625)
    q[mid & (qf >= 1.0)] = ml_dtypes.float8_e4m3(0.9375)
    q[mid & (qf <= -1.0)] = ml_dtypes.float8_e4m3(-0.9375)
    q[(s32 < -1.0) & (qf >= -1.0)] = ml_dtypes.float8_e4m3(-1.0625)
    wTall = q                                             # [IN, OUT] fp8
    mem = np.asarray(membrane_potential, np.float32)
    thr = np.asarray(adaptive_threshold, np.float32)
    refr = np.asarray(refractory_count, np.float32)

    in_maps = []
    for c in range(PB * QO):
        bi, oj = divmod(c, QO)
        # spikes: [NT, 128p, KT, NB] n-block-major, contiguous per block
        blkB = spike8[bi * BS:(bi + 1) * BS, :]           # [BS, IN]
        sph = blkB.reshape(NT, NB, KT, 128).transpose(0, 3, 2, 1)
        sph = np.ascontiguousarray(sph).reshape(NT * 128, KT * NB)
        # weights: pair-PAIR-major [KT2//2, 128p, 2q, MT, 2j, 128o],
        # m-major per pair (2KB contiguous per partition row)
        wt = wTall[:, oj * OS:(oj + 1) * OS]              # [IN, OS]
        wth = wt.reshape(KT2, 2, 128, MT, 128).transpose(0, 2, 3, 1, 4)
        wth = wth.reshape(KT2, 128, PAIRW)
        wth = wth.reshape(KT2 // 2, 2, 128, PAIRW).transpose(0, 2, 1, 3)
        wth = np.ascontiguousarray(wth).reshape((KT2 // 2) * 128, 2 * PAIRW)
        nvec = np.concatenate(
            [
                mem[oj * OS:(oj + 1) * OS].reshape(MT, 128).T,
                thr[oj * OS:(oj + 1) * OS].reshape(MT, 128).T,
                refr[oj * OS:(oj + 1) * OS].reshape(MT, 128).T,
            ],
            axis=1,
        )
        in_maps.append({
            "spikeT": sph,
            "wT": wth,
            "nvec": np.ascontiguousarray(nvec).astype(np.float32),
        })

    trace = bool(os.environ.get("KERNEL_PROFILE"))
    res = run_bass_kernel_spmd(
        nc, in_maps, core_ids=list(range(PB * QO)), trace=trace
    )
    LAST_EXEC_TIME_NS = res.exec_time_ns
    LAST_TRACE = getattr(res, "instructions_and_trace", None)

    spikes = np.empty((B, OUT), np.float32)
    for c in range(PB * QO):
        bi, oj = divmod(c, QO)
        blocks = res.results[c]["out_u8"].reshape(MT, NT, 128, NB)
        o = blocks.transpose(0, 2, 1, 3).reshape(OS, BS)  # [OS, BS]
        spikes[bi * BS:(bi + 1) * BS, oj * OS:(oj + 1) * OS] = o.T
    return spikes
